# revision 17
# baseline (speedup 1.0000x reference)
"""PINN Navier-Stokes residual kernel for trn2 (8 cores, data parallel), v5.

13-stream Taylor jet through the 3-128x8-2 tanh MLP.  Key structure:
  - All addend scale factors (3, 2, layer-0 c0 columns) are folded into
    pre-scaled weight copies host-side (W1C, Wh variants, W8G), so the
    per-layer elementwise work is exactly: 13 PSUM drains (Scalar),
    7 batched broadcast tensor_tensor products + 6 helper products
    (Vector), and a 5-op tanh chain.
  - Products are batched: one DVE op per chain factor covering all its
    consumers via a step-0 broadcast AP over a contiguous zc/zd tile.
  - A dedicated spare PSUM bank receives tiny keep-warm matmuls (paced
    by drain completions) so the PE's HAM clock gate stays at 8/8.
v5 changes vs v4:
  - Startup: points DMA for blocks 0/1 issued before the bulk weight
    DMAs so the first matmul starts ~2us in instead of ~36us.
  - Layer 8 uses 4-way PE column tiling (tile_position) so the 28
    M=16 addend matmuls run ~4x concurrent instead of serialized.
  - The final psi-jet -> (u,v,f_u,f_v) algebra runs per block (rows
    are per-block disjoint) split across Vector/GpSimd/Scalar, hiding
    it under the next block's compute instead of a ~23us serial tail.
"""

import os
import numpy as np
from contextlib import ExitStack

import concourse.bass as bass
import concourse.bacc as bacc
import concourse.tile as tile
from concourse import mybir
from concourse.bass_utils import run_bass_kernel_spmd

F32 = mybir.dt.float32
F16 = mybir.dt.float16
OP = mybir.AluOpType
AF = mybir.ActivationFunctionType

KEEP_WARM = os.environ.get("KEEP_WARM", "1") == "1"
N_CORES = 8
N = 32768
NLOC = N // N_CORES      # 4096 points per core
BLK = 1024
NBLK = NLOC // BLK       # 4
CH = 512
NCH = BLK // CH          # 2
H = 128
PB = NLOC // H           # 32

STREAMS = ["v", "x", "y", "t", "xx", "xy", "yy", "xt", "yt",
           "xxx", "xxy", "xyy", "yyy"]
ZC6 = ("x", "y", "t", "xx", "xy", "yy")
ZD6 = ("xt", "yt", "xxx", "xxy", "xyy", "yyy")
# (stream, addend) -> (weight_variant, source_tile_key, slot)
# weight variants: "1" = W, "m2" = -2W, "m4" = -4W, "m6" = -6W
# (AEX/AEY hold s*s1*z products; the -2 of sigma'' lives in the weights)
ADDENDS = {
    "v":   [("1", "S", 0)],
    "x":   [("1", "A6C", 0)],
    "y":   [("1", "A6C", 1)],
    "t":   [("1", "A6C", 2)],
    "xx":  [("m2", "AEX", 0), ("1", "A6C", 3)],
    "xy":  [("m2", "AEX", 1), ("1", "A6C", 4)],
    "yy":  [("m2", "AEY", 0), ("1", "A6C", 5)],
    "xt":  [("m2", "AEX", 2), ("1", "AZD", 0)],
    "yt":  [("m2", "AEY", 1), ("1", "AZD", 1)],
    "xxx": [("1", "AFX", 0), ("m6", "AEX", 3), ("1", "AZD", 2)],
    "xxy": [("1", "AFX", 1), ("m2", "AEY", 2), ("m4", "AEX", 4),
            ("1", "AZD", 3)],
    "xyy": [("1", "AFY", 0), ("m2", "AEX", 5), ("m4", "AEY", 3),
            ("1", "AZD", 4)],
    "yyy": [("1", "AFY", 1), ("m6", "AEY", 4), ("1", "AZD", 5)],
}
VARIANTS = ("1", "m2", "m4", "m6")
# layer-1 consumes layer-0 chain tiles directly through c0-scaled W1:
# stream -> (chain source, c0 column or None)
L1_SRC = {"v": ("s", None), "x": ("s1", 0), "y": ("s1", 1), "t": ("s1", 2),
          "xx": ("s2", 3), "xy": ("s2", 4), "yy": ("s2", 5),
          "xt": ("s2", 6), "yt": ("s2", 7),
          "xxx": ("s3h", 8), "xxy": ("s3h", 9), "xyy": ("s3h", 10),
          "yyy": ("s3h", 11)}

# layer-8 PE column-tiling: stream -> (col group, group-local one-hot row).
# Groups balanced by addend count: 4 / 8 / 9 / 7 matmuls per chunk.
_L8_G = [["v", "x", "y", "t"], ["xx", "xy", "yy", "xt"],
         ["yt", "xxx", "xxy"], ["xyy", "yyy"]]
L8_GROUP = {s: (g, r) for g, ss in enumerate(_L8_G) for r, s in enumerate(ss)}


def _build():
    nc = bacc.Bacc(None, target_bir_lowering=False)

    pts_d = nc.declare_dram_parameter("pts", [3, NLOC], F32, False)
    w0_d = nc.declare_dram_parameter("W0f", [3, H], F32, False)
    wh_d = {li: nc.declare_dram_parameter(f"Whall_{li}", [H, 4 * H], F16,
                                          False)
            for li in range(2, 8)}
    w1c_d = nc.declare_dram_parameter("W1C", [H, 13 * H], F16, False)
    b_d = nc.declare_dram_parameter("ball", [H, 8], F32, False)
    w8g_d = nc.declare_dram_parameter("W8Gall", [H, 4 * 13 * 32], F16,
                                      False)
    b8_d = nc.declare_dram_parameter("b8v", [H, 1], F32, False)
    lam_d = nc.declare_dram_parameter("lam", [H, 3], F32, False)
    cm23_d = nc.declare_dram_parameter("cm23", [H, 1], F32, False)
    out_d = {k: nc.declare_dram_parameter(k, [H, PB], F32, True)
             for k in ["uo", "vo", "fuo", "fvo"]}

    with tile.TileContext(nc) as tc, ExitStack() as ctx:
        cpool = ctx.enter_context(tc.tile_pool(name="consts", bufs=1))
        apool = ctx.enter_context(tc.tile_pool(name="A", bufs=1))
        zcp = ctx.enter_context(tc.tile_pool(name="zc", bufs=int(os.environ.get("ZCB","2"))))
        chain = ctx.enter_context(tc.tile_pool(name="chain", bufs=int(os.environ.get("CHB","3"))))
        misc = ctx.enter_context(tc.tile_pool(name="misc", bufs=1))
        fpool = ctx.enter_context(tc.tile_pool(name="fin", bufs=1))
        zpool = ctx.enter_context(
            tc.tile_pool(name="psum_z", bufs=3, space="PSUM"))
        z8pool = ctx.enter_context(
            tc.tile_pool(name="psum_z8", bufs=1, space="PSUM"))
        dpool = ctx.enter_context(
            tc.tile_pool(name="psum_dummy", bufs=1, space="PSUM"))

        def ctile(name, shape, dt):
            return cpool.tile(shape, dt, name=name, tag=name)

        # Startup DMAs: first matmul needs only w0s + biases + points.
        # Consolidated tensors (variants packed) keep the descriptor count
        # low; the heavy late-layer weights go on the gpsimd queue so both
        # queues fill SBUF in parallel.
        w0s = ctile("w0s", [3, H], F32)
        nc.sync.dma_start(w0s[:], w0_d[:])
        ball = ctile("ball", [H, 8], F32)
        nc.sync.dma_start(ball[:], b_d[:])
        bss = {li: ball[:, li:li + 1] for li in range(8)}
        ptsb_pre = []
        for blk in range(min(2, NBLK)):
            pt = misc.tile([3, BLK], F32, name="ptsb", tag="ptsb", bufs=2)
            nc.sync.dma_start(pt[:], pts_d[:, bass.ts(blk, BLK)])
            ptsb_pre.append(pt)
        w8gall = ctile("w8gall", [H, 4 * 13 * 32], F16)
        nc.gpsimd.dma_start(w8gall[:], w8g_d[:])
        w8gs = {v: w8gall[:, vi * 13 * 32:(vi + 1) * 13 * 32]
                for vi, v in enumerate(VARIANTS)}
        w1cs = ctile("w1cs", [H, 13 * H], F16)
        nc.sync.dma_start(w1cs[:], w1c_d[:])
        whall = {}
        whs = {}
        for li in range(2, 8):
            whall[li] = ctile(f"whall_{li}", [H, 4 * H], F16)
            eng = nc.sync if li < 5 else nc.gpsimd
            eng.dma_start(whall[li][:], wh_d[li][:])
            for vi, v in enumerate(VARIANTS):
                whs[(li, v)] = whall[li][:, vi * H:(vi + 1) * H]
        b8s = ctile("b8s", [H, 1], F32)
        nc.sync.dma_start(b8s[:], b8_d[:])
        lams = ctile("lams", [H, 3], F32)
        nc.sync.dma_start(lams[:], lam_d[:])
        cm23 = ctile("cm23", [H, 1], F32)
        nc.sync.dma_start(cm23[:], cm23_d[:])

        z8stage = misc.tile([H, NLOC], F16, name="z8stage", tag="z8stage")

        V, S, G = nc.vector, nc.scalar, nc.gpsimd

        def chain_tiles(need_s2=False):
            c = {}
            c["s"] = chain.tile([H, BLK], F16, name="cs", tag="cs")
            c["t1"] = chain.tile([H, BLK], F16, name="ct1", tag="ct1",
                                 bufs=1)
            c["s1"] = chain.tile([H, BLK], F16, name="cs1", tag="cs1")
            c["w3"] = chain.tile([H, BLK], F16, name="cw3", tag="cw3",
                                 bufs=1)
            if need_s2:
                c["s2"] = chain.tile([H, BLK], F16, name="cs2", tag="cs2")
            c["s3h"] = chain.tile([H, BLK], F16, name="cs3h", tag="cs3h")
            return c

        def chain_ops(li, ct, zt, need_s2=False):
            S.activation(ct["s"][:], zt[:], AF.Tanh, bias=bss[li])
            S.activation(ct["t1"][:], ct["s"][:], AF.Square)
            V.tensor_scalar(ct["s1"][:], ct["t1"][:], -1.0, 1.0,
                            OP.mult, OP.add)
            if need_s2:
                # s2m = s*s1 (the -2 of sigma'' lives in the scaled weights)
                V.tensor_tensor(ct["s2"][:], ct["s"][:], ct["s1"][:],
                                OP.mult)
            S.activation(ct["w3"][:], ct["t1"][:], AF.Square,
                         bias=cm23[:])
            V.tensor_scalar(ct["s3h"][:], ct["w3"][:], -6.0, 2.0 / 3.0,
                            OP.mult, OP.add)

        def bprod2(out_tile, fac_ap, zt6, lo, hi):
            k = hi - lo
            dst = out_tile[:, 0:k * BLK].rearrange("p (s f) -> p s f", s=k)
            src1 = zt6[:, lo * BLK:hi * BLK].rearrange(
                "p (s f) -> p s f", s=k)
            src0 = fac_ap.unsqueeze(1).broadcast_to([H, k, BLK])
            V.tensor_tensor(dst, src0, src1, OP.mult)

        def bprod(out_tile, fac, zt6, lo, hi):
            # out[:, lo*BLK:hi*BLK] = fac (broadcast) * zt6[:, lo*BLK:hi*BLK]
            k = hi - lo
            dst = out_tile[:, lo * BLK:hi * BLK].rearrange(
                "p (s f) -> p s f", s=k)
            src1 = zt6[:, lo * BLK:hi * BLK].rearrange(
                "p (s f) -> p s f", s=k)
            src0 = fac[:].unsqueeze(1).broadcast_to([H, k, BLK])
            V.tensor_tensor(dst, src0, src1, OP.mult)

        def mm_addends(zt, adds):
            for c in range(NCH):
                csl = bass.ts(c, CH)
                for j, (lhsT, rhs) in enumerate(adds):
                    nc.tensor.matmul(zt[:, csl], lhsT, rhs[:, csl],
                                     start=(j == 0),
                                     stop=(j == len(adds) - 1))

        def keep_warm(lhsT, rhs_tile, off):
            # tiny matmul into the dedicated scratch bank; paced by the
            # availability of rhs_tile (a freshly drained tile)
            nc.tensor.matmul(dummy_ps[0:16, 0:16], lhsT[:, 0:16],
                             rhs_tile[:, off:off + 16],
                             start=True, stop=True, skip_group_check=True)

        dummy_ps = dpool.tile([H, CH], F32, name="dummy", tag="dummy")

        def hidden_mms(li, A_prev, warm_w):
            """Matmul + drain + tanh-chain phase of a hidden layer.
            A_prev: dict stream -> list of (lhsT AP, rhs AP-tile)."""
            ct = chain_tiles()
            zc6 = zcp.tile([H, 6 * BLK], F16, name="zc6", tag="zc6")
            zd6 = zcp.tile([H, 6 * BLK], F16, name="zd6", tag="zd6")
            warm_targets = []
            for si, s in enumerate(STREAMS):
                zt = zpool.tile([H, BLK], F32, name=f"z_{s}", tag="z")
                mm_addends(zt, A_prev[s])
                if s == "v":
                    chain_ops(li, ct, zt)
                elif s in ZC6:
                    k = ZC6.index(s)
                    S.activation(zc6[:, k * BLK:(k + 1) * BLK], zt[:],
                                 AF.Copy)
                    warm_targets.append((zc6, k * BLK))
                else:
                    k = ZD6.index(s)
                    S.activation(zd6[:, k * BLK:(k + 1) * BLK], zt[:],
                                 AF.Copy)
                    warm_targets.append((zd6, k * BLK))
            # keep-warm dummies AFTER all real matmuls of this layer: each
            # fires when its drain completes, spreading tiny PE activity
            # across the products phase so HAM never sees an idle window.
            if KEEP_WARM:
                for wt, off in warm_targets:
                    keep_warm(warm_w, wt, off)
            return ct, zc6, zd6

        def hidden_products(li, ct, zc6, zd6, warm_w):
            """Product phase: builds the next layer's addend tiles."""
            hp = {}
            exy = misc.tile([H, 2 * BLK], F16, name="exy", tag="exy", bufs=2)
            pxy2 = misc.tile([H, 2 * BLK], F16, name="pxy2", tag="pxy2",
                             bufs=1)
            fxy = misc.tile([H, 2 * BLK], F16, name="fxy", tag="fxy", bufs=2)
            S.activation(pxy2[:, 0:BLK], zc6[:, 0:BLK], AF.Square)
            S.activation(pxy2[:, BLK:2 * BLK], zc6[:, BLK:2 * BLK],
                         AF.Square)
            # A6C[0:2] early so exy can reuse it: ex = s*(s1*z_x) = s2m*z_x
            A6C = apool.tile([H, 6 * BLK], F16, name="A6C", tag="A6C")
            bprod(A6C, ct["s1"], zc6, 0, 2)
            bprod(exy, ct["s"], A6C, 0, 2)
            bprod(fxy, ct["s3h"], pxy2, 0, 2)
            hp["ex"] = exy[:, 0:BLK]
            hp["ey"] = exy[:, BLK:2 * BLK]
            hp["fx"] = fxy[:, 0:BLK]
            hp["fy"] = fxy[:, BLK:2 * BLK]
            # batched addend products
            AEX = apool.tile([H, 6 * BLK], F16, name="AEX", tag="AEX")
            AEY = apool.tile([H, 5 * BLK], F16, name="AEY", tag="AEY")
            AFX = apool.tile([H, 2 * BLK], F16, name="AFX", tag="AFX")
            AFY = apool.tile([H, 2 * BLK], F16, name="AFY", tag="AFY")
            AZD = apool.tile([H, 6 * BLK], F16, name="AZD", tag="AZD")
            bprod(A6C, ct["s1"], zc6, 2, 6)
            bprod2(AEX, hp["ex"], zc6, 0, 6)
            # AEY covers zc6 slots 1..5 -> own slots 0..4
            dst = AEY[:].rearrange("p (s f) -> p s f", s=5)
            src1 = zc6[:, BLK:6 * BLK].rearrange("p (s f) -> p s f", s=5)
            V.tensor_tensor(dst, hp["ey"].unsqueeze(1).broadcast_to(
                [H, 5, BLK]), src1, OP.mult)
            bprod2(AFX, hp["fx"], zc6, 0, 2)
            bprod2(AFY, hp["fy"], zc6, 0, 2)
            # AZD split in two for earlier availability of xt/yt
            bprod(AZD, ct["s1"], zd6, 0, 2)
            bprod(AZD, ct["s1"], zd6, 2, 6)
            if KEEP_WARM:
                keep_warm(warm_w, A6C, 0)
                keep_warm(warm_w, AZD, 0)
            tiles = {"S": ct["s"], "A6C": A6C, "AEX": AEX, "AEY": AEY,
                     "AFX": AFX, "AFY": AFY, "AZD": AZD}

            def ap_of(src, slot):
                t = tiles[src]
                if src == "S":
                    return t[:]
                return t[:, slot * BLK:(slot + 1) * BLK]

            A_new = {}
            for s, adds in ADDENDS.items():
                if li < 7:
                    W = {v: whs[(li + 1, v)] for v in VARIANTS}
                    A_new[s] = [(W[v], ap_of(src, slot))
                                for v, src, slot in adds]
                else:
                    si = STREAMS.index(s)
                    A_new[s] = [(w8gs[v][:, 32 * si:32 * si + 32],
                                 ap_of(src, slot))
                                for v, src, slot in adds]
            return A_new

        def hidden_layer(li, A_prev, warm_w):
            ct, zc6, zd6 = hidden_mms(li, A_prev, warm_w)
            return hidden_products(li, ct, zc6, zd6, warm_w)

        def layer0(blk, pre=None):
            if pre is None:
                ptsb = misc.tile([3, BLK], F32, name="ptsb", tag="ptsb",
                                 bufs=2)
                nc.sync.dma_start(ptsb[:], pts_d[:, bass.ts(blk, BLK)])
            else:
                ptsb = pre
            ct = chain_tiles(need_s2=True)
            zt = zpool.tile([H, BLK], F32, name="z0", tag="z")
            for c in range(NCH):
                csl = bass.ts(c, CH)
                nc.tensor.matmul(zt[:, csl], w0s[:], ptsb[:, csl],
                                 start=True, stop=True)
            chain_ops(0, ct, zt, need_s2=True)
            A_new = {}
            for s in STREAMS:
                cn, col = L1_SRC[s]
                wsl = w1cs[:, 0 * H:H] if col is None else \
                    w1cs[:, (col + 1) * H:(col + 2) * H]
                A_new[s] = [(wsl, ct[cn])]
            return A_new

        def layer8(blk, A_prev):
            # 4-way column-tiled: group g's addends accumulate rows
            # [32g, 32g+32) of one [128, CH] PSUM tile; groups run
            # concurrently on the PE's col-group sub-arrays.
            per_group = [[] for _ in range(4)]
            for s in STREAMS:
                g, _r = L8_GROUP[s]
                for lhsT, rhs in A_prev[s]:
                    per_group[g].append((lhsT, rhs))
            counts = [len(pg) for pg in per_group]
            for c in range(NCH):
                csl = bass.ts(c, CH)
                z8 = z8pool.tile([H, CH], F32, name="z8", tag="z8")
                # groups must run sequentially: PSUM accumulation start
                # clears has_written for the whole zero region, so only
                # one accumulation lifetime per bank at a time.
                for g in range(4):
                    for k, (lhsT, rhs) in enumerate(per_group[g]):
                        nc.tensor.matmul(z8[32 * g:32 * g + 32, :], lhsT,
                                         rhs[:, csl],
                                         start=(k == 0),
                                         stop=(k == counts[g] - 1),
                                         tile_position=(0, 32 * g),
                                         skip_group_check=True)
                S.activation(z8stage[:, bass.ts(blk * NCH + c, CH)],
                             z8[:], AF.Copy)

        def ft(name, dt=F16):
            return fpool.tile([H, PB], dt, name=name, tag=name)

        Z = {s: ft(f"Z_{s}") for s in STREAMS}

        # ---------------- final fp32 jet -> outputs (per block) --------
        # Block blk's 1024 points are ROWS [32*blk, 32*blk+32) of the
        # final tiles, so each block's jet is row-disjoint and can run
        # under the next block's layer compute.  Ops are split between
        # Vector (x-half), GpSimd (y-half) and Scalar (tanh chain).
        JT = {}
        for nm in ["s8", "t18", "s18", "w38", "s28", "s38", "e8x", "e8y",
                   "p8xx", "p8yy", "f8x", "f8y",
                   "p_xx_a", "p_xx_b", "p_xx", "p_xy_a", "p_xy_b", "p_xy",
                   "p_yy_a", "p_yy_b", "p_yy", "p_xt_a", "p_xt_b", "p_xt",
                   "p_yt_a", "p_yt_b", "p_yt",
                   "x3_a", "x3_b", "x3_c", "x3_ab", "p_xxx",
                   "y3_a", "y3_b", "y3_c", "y3_ab", "p_yyy",
                   "m1_a", "m1_b", "m1_c", "m1_d", "m1_ab", "m1_cd",
                   "p_xxy",
                   "m2_a", "m2_b", "m2_c", "m2_d", "m2_ab", "m2_cd",
                   "p_xyy",
                   "fu_a", "fu_b", "fu_ab", "fu_l", "fu_c",
                   "fv_a", "fv_b", "fv_ab", "fv_l", "fv_c"]:
            JT[nm] = ft(nm)
        for nm in ["u", "vv", "f_u", "f_v"]:
            JT[nm] = ft(nm, F32)

        def jet(blk):
            R = slice(32 * blk, 32 * (blk + 1))
            t = JT

            def vt(d, a, b, op=OP.mult):
                V.tensor_tensor(t[d][R, :], a[R, :], b[R, :], op)

            def gt(d, a, b, op=OP.mult):
                G.tensor_tensor(t[d][R, :], a[R, :], b[R, :], op)

            S.activation(t["s8"][R, :], Z["v"][R, :], AF.Tanh,
                         bias=b8s[R, :])
            S.activation(t["t18"][R, :], t["s8"][R, :], AF.Square)
            S.activation(t["s18"][R, :], t["t18"][R, :], AF.Copy,
                         bias=1.0, scale=-1.0)
            S.activation(t["w38"][R, :], t["t18"][R, :], AF.Copy,
                         bias=-2.0, scale=6.0)
            V.scalar_tensor_tensor(t["s28"][R, :], t["s8"][R, :], -2.0,
                                   t["s18"][R, :], OP.mult, OP.mult)
            gt("s38", t["w38"], t["s18"])
            vt("e8x", t["s28"], Z["x"])
            gt("e8y", t["s28"], Z["y"])
            vt("p8xx", Z["x"], Z["x"])
            gt("p8yy", Z["y"], Z["y"])
            vt("f8x", t["s38"], t["p8xx"])
            gt("f8y", t["s38"], t["p8yy"])
            V.tensor_tensor(t["u"][R, :], t["s18"][R, :], Z["y"][R, :],
                            OP.mult)
            V.scalar_tensor_tensor(t["vv"][R, :], t["s18"][R, :], -1.0,
                                   Z["x"][R, :], OP.mult, OP.mult)
            # second derivatives
            gt("p_xx_a", t["e8x"], Z["x"])
            gt("p_xx_b", t["s18"], Z["xx"])
            gt("p_xx", t["p_xx_a"], t["p_xx_b"], OP.add)
            vt("p_xy_a", t["e8x"], Z["y"])
            vt("p_xy_b", t["s18"], Z["xy"])
            vt("p_xy", t["p_xy_a"], t["p_xy_b"], OP.add)
            gt("p_yy_a", t["e8y"], Z["y"])
            gt("p_yy_b", t["s18"], Z["yy"])
            gt("p_yy", t["p_yy_a"], t["p_yy_b"], OP.add)
            vt("p_xt_a", t["e8x"], Z["t"])
            vt("p_xt_b", t["s18"], Z["xt"])
            vt("p_xt", t["p_xt_a"], t["p_xt_b"], OP.add)
            gt("p_yt_a", t["e8y"], Z["t"])
            gt("p_yt_b", t["s18"], Z["yt"])
            gt("p_yt", t["p_yt_a"], t["p_yt_b"], OP.add)
            # third derivatives, pure (xxx / yyy)
            gt("x3_a", t["f8x"], Z["x"])
            V.scalar_tensor_tensor(t["x3_b"][R, :], t["e8x"][R, :], 3.0,
                                   Z["xx"][R, :], OP.mult, OP.mult)
            gt("x3_c", t["s18"], Z["xxx"])
            gt("x3_ab", t["x3_a"], t["x3_b"], OP.add)
            gt("p_xxx", t["x3_ab"], t["x3_c"], OP.add)
            gt("y3_a", t["f8y"], Z["y"])
            V.scalar_tensor_tensor(t["y3_b"][R, :], t["e8y"][R, :], 3.0,
                                   Z["yy"][R, :], OP.mult, OP.mult)
            gt("y3_c", t["s18"], Z["yyy"])
            gt("y3_ab", t["y3_a"], t["y3_b"], OP.add)
            gt("p_yyy", t["y3_ab"], t["y3_c"], OP.add)
            # third derivatives, mixed (xxy / xyy)
            vt("m1_a", t["f8x"], Z["y"])
            vt("m1_b", t["e8y"], Z["xx"])
            V.scalar_tensor_tensor(t["m1_c"][R, :], t["e8x"][R, :], 2.0,
                                   Z["xy"][R, :], OP.mult, OP.mult)
            vt("m1_d", t["s18"], Z["xxy"])
            vt("m1_ab", t["m1_a"], t["m1_b"], OP.add)
            vt("m1_cd", t["m1_c"], t["m1_d"], OP.add)
            vt("p_xxy", t["m1_ab"], t["m1_cd"], OP.add)
            gt("m2_a", t["f8y"], Z["x"])
            gt("m2_b", t["e8x"], Z["yy"])
            V.scalar_tensor_tensor(t["m2_c"][R, :], t["e8y"][R, :], 2.0,
                                   Z["xy"][R, :], OP.mult, OP.mult)
            gt("m2_d", t["s18"], Z["xyy"])
            gt("m2_ab", t["m2_a"], t["m2_b"], OP.add)
            gt("m2_cd", t["m2_c"], t["m2_d"], OP.add)
            gt("p_xyy", t["m2_ab"], t["m2_cd"], OP.add)
            # residuals
            vt("fu_a", t["u"], t["p_xy"])
            vt("fu_b", t["vv"], t["p_yy"])
            vt("fu_ab", t["fu_a"], t["fu_b"], OP.add)
            V.scalar_tensor_tensor(t["fu_l"][R, :], t["fu_ab"][R, :],
                                   lams[R, 0:1], t["p_yt"][R, :],
                                   OP.mult, OP.add)
            vt("fu_c", t["p_xxy"], t["p_yyy"], OP.add)
            V.scalar_tensor_tensor(t["f_u"][R, :], t["fu_c"][R, :],
                                   lams[R, 1:2], t["fu_l"][R, :],
                                   OP.mult, OP.add)
            gt("fv_a", t["u"], t["p_xx"])
            gt("fv_b", t["vv"], t["p_xy"])
            gt("fv_ab", t["fv_a"], t["fv_b"], OP.add)
            V.scalar_tensor_tensor(t["fv_l"][R, :], t["fv_ab"][R, :],
                                   lams[R, 0:1], t["p_xt"][R, :],
                                   OP.mult, OP.add)
            gt("fv_c", t["p_xxx"], t["p_xyy"], OP.add)
            V.scalar_tensor_tensor(t["f_v"][R, :], t["fv_c"][R, :],
                                   lams[R, 2:3], t["fv_l"][R, :],
                                   OP.mult, OP.subtract)
            nc.sync.dma_start(out_d["uo"][R, :], t["u"][R, :])
            nc.sync.dma_start(out_d["vo"][R, :], t["vv"][R, :])
            nc.sync.dma_start(out_d["fuo"][R, :], t["f_u"][R, :])
            nc.sync.dma_start(out_d["fvo"][R, :], t["f_v"][R, :])

        # Software pipeline across the block boundary: the next block's
        # layer 0 + layer 1 matmul/drain phase is issued before this
        # block's layer-8 matmul burst, so Scalar/Vector stay fed while
        # the PE grinds through layer 8, and layer 8's PE work overlaps
        # the next block's product phase.
        A = hidden_layer(1, layer0(0, ptsb_pre[0]), warm_w=w1cs)
        for blk in range(NBLK):
            for li in range(2, 8):
                A = hidden_layer(li, A, warm_w=w1cs)
            pre = None
            if blk + 1 < NBLK:
                A0n = layer0(blk + 1,
                             ptsb_pre[blk + 1]
                             if blk + 1 < len(ptsb_pre) else None)
                pre = hidden_mms(1, A0n, w1cs)
            layer8(blk, A)
            if pre is not None:
                A = hidden_products(1, *pre, w1cs)
            # stream this block's psi-jet values into the final tiles.
            # Point p maps to Z[p // PB, p % PB]; block blk's 1024 points
            # are ROWS [32*blk, 32*blk+32) of the final tiles.
            for si, s in enumerate(STREAMS):
                g, r = L8_GROUP[s]
                row = 32 * g + r
                eng = nc.gpsimd if si % 2 == 0 else nc.sync
                eng.dma_start(
                    Z[s][32 * blk:32 * (blk + 1), :],
                    z8stage[row:row + 1, bass.ts(blk, BLK)])
            jet(blk)

    return nc


_CACHE = {}


def _get_nc():
    if "nc" not in _CACHE:
        nc = _build()
        nc.finalize()
        _CACHE["nc"] = nc
    return _CACHE["nc"]


def prep_shared(inputs):
    f32 = np.float32
    f16 = np.float16
    x = np.asarray(inputs["x"], f32)[:, 0]
    y = np.asarray(inputs["y"], f32)[:, 0]
    t = np.asarray(inputs["t"], f32)[:, 0]
    pts = np.ascontiguousarray(np.stack([x, y, t], 0))          # [3, N]
    W0 = np.asarray(inputs["W0"], f32)
    cx, cy, ct = W0[0], W0[1], W0[2]
    c0 = np.stack(
        [cx, cy, ct,
         cx * cx, cx * cy, cy * cy, cx * ct, cy * ct,
         cx ** 3, cx * cx * cy, cx * cy * cy, cy ** 3], 1).astype(f32)
    W1 = np.asarray(inputs["W1"], f32)
    w1c = np.zeros([H, 13 * H], f32)
    w1c[:, 0:H] = W1
    for col in range(12):
        sc = -2.0 if 3 <= col <= 7 else 1.0
        w1c[:, (col + 1) * H:(col + 2) * H] = sc * W1 * c0[:, col:col + 1]
    w8 = np.asarray(inputs["W8"], f32)[:, 0]
    lam1 = f32(np.asarray(inputs["lam1"]).reshape(-1)[0])
    lam2 = f32(np.asarray(inputs["lam2"]).reshape(-1)[0])
    shared = {
        "W0f": np.ascontiguousarray(W0),
        "W1C": w1c.astype(f16),
        "b8v": np.full([H, 1], np.asarray(inputs["b8"]).reshape(-1)[0], f32),
        "lam": np.tile(np.array([[lam1, -lam2, lam2]], f32), (H, 1)),
        "cm23": np.full([H, 1], -2.0 / 3.0, f32),
    }
    VSC = {"1": 1.0, "m2": -2.0, "m4": -4.0, "m6": -6.0}
    w8g_parts = []
    for v, sc in VSC.items():
        W8G = np.zeros([H, 13 * 32], f16)
        for si, s in enumerate(STREAMS):
            g, r = L8_GROUP[s]
            W8G[:, 32 * si + r] = (sc * w8).astype(f16)
        w8g_parts.append(W8G)
    shared["W8Gall"] = np.ascontiguousarray(np.concatenate(w8g_parts, 1))
    for li in range(2, 8):
        Wl = np.asarray(inputs[f"W{li}"], f32)
        shared[f"Whall_{li}"] = np.ascontiguousarray(np.concatenate(
            [(sc * Wl).astype(f16) for v, sc in VSC.items()], 1))
    shared["ball"] = np.ascontiguousarray(np.stack(
        [np.asarray(inputs[f"b{li}"], f32).reshape(H) for li in range(8)],
        1))
    return shared, pts


def kernel(**inputs):
    nc = _get_nc()
    f32 = np.float32
    shared, pts = prep_shared(inputs)

    in_maps = []
    for c in range(N_CORES):
        m = dict(shared)
        m["pts"] = np.ascontiguousarray(pts[:, c * NLOC:(c + 1) * NLOC])
        in_maps.append(m)

    trace = bool(os.environ.get("BASS_KERNEL_TRACE"))
    tdir = os.environ.get("BASS_KERNEL_TRACE_DIR") or None
    res = run_bass_kernel_spmd(nc, in_maps, list(range(N_CORES)),
                               trace=trace, tmpdir=tdir)
    kernel.last_exec_time_ns = res.exec_time_ns
    outs = []
    for name in ["uo", "vo", "fuo", "fvo"]:
        full = np.concatenate(
            [np.asarray(res.results[c][name], f32).reshape(-1)
             for c in range(N_CORES)])
        outs.append(full[:, None])
    return tuple(outs)


kernel.last_exec_time_ns = None
